# revision 7
# baseline (speedup 1.0000x reference)
"""Trainium2 Bass kernel for nn_CtcHead (segment-reduce + 2-layer head + CE).

Sharding: 8 cores, core c -> (batch b = c//2, half h = c%2).
Key algebraic reduction: cell_states[b,s] = group_mean[b, indicator[b,s]]
with only 256 groups per batch, so the tanh-head/log-softmax/argmax run on
256 group rows per batch instead of 16384 tokens.  Per-token outputs are
gathers from a [256, 9] table via a one-hot matmul.

Matmuls run as fp32r (fp32 with 11-bit mantissa, round-to-nearest-even,
exact PSUM f32 accumulate).  Host pre-rounds float inputs to the fp32r
grid so tiles can be DMA'd directly into float32r SBUF tensors.
"""
import sys
import numpy as np

sys.path.insert(0, "/opt/trn_rl_repo")

B, S, H = 4, 4096, 768
G = 256          # indicator groups
T = 8            # NUM_TYPES
HALF = S // 2    # tokens per core
HT = HALF // 128  # token s-blocks per core (16)
HB = H // 128    # h-blocks (6)
N_CORES = 8

_CACHE = {}
TRACE = False


def _rnd_fp32r(x):
    """Round float32 array to the fp32r grid (11-bit mantissa, nearest-even)."""
    u = np.ascontiguousarray(x, np.float32).view(np.uint32).astype(np.uint64)
    lsb = (u >> np.uint64(12)) & np.uint64(1)
    u2 = (u + np.uint64(0x7FF) + lsb) & np.uint64(0xFFFFF000)
    return u2.astype(np.uint32).view(np.float32)


def _build_nc(nk):
    """Build the Bass program.  nk = number of 128-token k-tiles fed to the
    segment-sum (32: full batch per core)."""
    import concourse.bacc as bacc
    import concourse.tile as tile
    import concourse.mybir as mybir

    F32 = mybir.dt.float32
    F32R = mybir.dt.float32r
    AF = mybir.ActivationFunctionType
    OP = mybir.AluOpType
    AX = mybir.AxisListType

    nc = bacc.Bacc("TRN2", target_bir_lowering=False, debug=False,
                   num_devices=N_CORES)

    # packed small constants, one DMA:
    # cols: indcol(nk) labcol(HT) iotab(G) iotag(2) iota8(T) rev7(T) bpb(T)
    #       emask(1) omask(1) invc(2) bu(HB) ident(128)
    CW = nk + HT + G + 2 + T + T + T + 1 + 1 + 2 + HB + 128
    d_e = nc.dram_tensor("e", [nk * 128, H], F32, kind="ExternalInput")
    d_cpk = nc.dram_tensor("cpk", [128, CW], F32, kind="ExternalInput")
    d_indrep = nc.dram_tensor("indrep", [128, HALF], F32, kind="ExternalInput")
    d_wut = nc.dram_tensor("wut", [H, H], F32, kind="ExternalInput")
    d_wpt = nc.dram_tensor("wpt", [H, T], F32, kind="ExternalInput")

    d_pred = nc.dram_tensor("out_pred", [128, HT], F32, kind="ExternalOutput")
    d_loss = nc.dram_tensor("out_loss", [128, 2], F32, kind="ExternalOutput")

    with tile.TileContext(nc) as tc:
        with (
            tc.tile_pool(name="const", bufs=1) as cp,
            tc.tile_pool(name="eb", bufs=1) as eb,
            tc.tile_pool(name="ob", bufs=4) as ob,
            tc.tile_pool(name="work", bufs=1) as wk,
            tc.tile_pool(name="small", bufs=2) as sm,
            tc.tile_pool(name="pseg", bufs=1, space="PSUM") as pseg,
            tc.tile_pool(name="ptr", bufs=2, space="PSUM") as ptr,
            tc.tile_pool(name="pmm", bufs=2, space="PSUM") as pmm,
        ):
            # ---- constants (one packed DMA + 3 weight DMAs)
            cpk = cp.tile([128, CW], F32, tag="cpk")
            nc.sync.dma_start(cpk[:], d_cpk.ap())
            o0 = 0
            indcol = cpk[:, o0:o0 + nk]; o0 += nk
            labcol = cpk[:, o0:o0 + HT]; o0 += HT
            iotab = cpk[:, o0:o0 + G]; o0 += G
            iotag = cpk[:, o0:o0 + 2]; o0 += 2
            iota8 = cpk[:, o0:o0 + T]; o0 += T
            rev7 = cpk[:, o0:o0 + T]; o0 += T
            bpb = cpk[:, o0:o0 + T]; o0 += T
            emask = cpk[:, o0:o0 + 1]; o0 += 1
            omask = cpk[:, o0:o0 + 1]; o0 += 1
            invc = [cpk[:, o0 + gb:o0 + gb + 1] for gb in range(2)]; o0 += 2
            bu = [cpk[:, o0 + j:o0 + j + 1] for j in range(HB)]; o0 += HB
            ident = cpk[:, o0:o0 + 128]; o0 += 128

            indrep = cp.tile([128, HALF], F32, tag="indrep")
            nc.gpsimd.dma_start(indrep[:], d_indrep.ap())
            wut_all = cp.tile([128, HB, H], F32R, tag="wut_all")
            nc.sync.dma_start(
                wut_all[:], d_wut.ap().rearrange("(j p) h -> p j h", p=128).bitcast(F32R))
            wut = [wut_all[:, j, :] for j in range(HB)]
            wpt_all = cp.tile([128, HB, T], F32R, tag="wpt_all")
            nc.sync.dma_start(
                wpt_all[:], d_wpt.ap().rearrange("(j p) t -> p j t", p=128).bitcast(F32R))
            wpt = [wpt_all[:, j, :] for j in range(HB)]

            # ---- phase 1: segment sums  (PSUM [g-block, 768] over nk k-tiles)
            psA = [pseg.tile([128, 512], F32, tag=f"psA{gb}", name=f"psA{gb}") for gb in range(2)]
            psB = [pseg.tile([128, 256], F32, tag=f"psB{gb}", name=f"psB{gb}") for gb in range(2)]
            CH = 4  # k-tiles per e-chunk
            echunks = []
            for c in range(nk // CH):
                e_c = eb.tile([128, CH, H], F32R, tag=f"e{c}", name=f"e{c}")
                eng = nc.gpsimd if (c % 2) else nc.sync
                eng.dma_start(
                    e_c[:],
                    d_e.ap()[c * CH * 128:(c + 1) * CH * 128, :]
                    .rearrange("(k p) h -> p k h", p=128).bitcast(F32R))
                echunks.append(e_c)
            for k in range(nk):
                e_t = echunks[k // CH][:, k % CH, :]
                o_t = ob.tile([128, G], F32R, tag="o")
                nc.vector.tensor_tensor(
                    o_t[:], indcol[:, k:k + 1].broadcast_to([128, G]), iotab[:],
                    op=OP.is_equal)
                st, sp = (k == 0), (k == nk - 1)
                for gb in range(2):
                    osl = o_t[:, gb * 128:(gb + 1) * 128]
                    nc.tensor.matmul(psA[gb][:], osl, e_t[:, 0:512], start=st, stop=sp)
                    nc.tensor.matmul(psB[gb][:], osl, e_t[:, 512:H], start=st, stop=sp)

            # ---- phase 2: means [g, hin] f32  (scale by 1/count)
            means = [wk.tile([128, H], F32, tag=f"means{gb}", name=f"means{gb}") for gb in range(2)]
            for gb in range(2):
                nc.scalar.mul(means[gb][:, 0:512], psA[gb][:], invc[gb][:])
                nc.scalar.mul(means[gb][:, 512:H], psB[gb][:], invc[gb][:])

            # ---- phase 3: transpose -> meansT [hin, g] F32R
            meansT = [wk.tile([128, G], F32R, tag=f"meansT{hb}", name=f"meansT{hb}") for hb in range(HB)]
            for hb in range(HB):
                for gb in range(2):
                    tp = ptr.tile([128, 128], F32, tag="tp")
                    nc.tensor.transpose(tp[:], means[gb][:, hb * 128:(hb + 1) * 128],
                                        ident[:])
                    nc.scalar.copy(meansT[hb][:, gb * 128:(gb + 1) * 128], tp[:])

            # ---- phase 4: L1  h1T[hout-block] = tanh(wut.T @ meansT + bu)
            h1T = [wk.tile([128, G], F32R, tag=f"h1T{j}", name=f"h1T{j}") for j in range(HB)]
            for j in range(HB):
                h1ps = pmm.tile([128, G], F32, tag="mm")
                for hb in range(HB):
                    nc.tensor.matmul(h1ps[:], wut[hb][:, j * 128:(j + 1) * 128],
                                     meansT[hb][:], start=(hb == 0), stop=(hb == HB - 1))
                nc.scalar.activation(h1T[j][:], h1ps[:], AF.Tanh, bias=bu[j][:])

            # ---- phase 5: L2 logits [g-block, 8] + softmax/argmax -> table
            table = [wk.tile([128, 2 + T], F32R, tag=f"table{gb}", name=f"table{gb}") for gb in range(2)]
            for gb in range(2):
                lps = pmm.tile([128, G], F32, tag="mm", name="lps")[:, 0:T]
                for j in range(HB):
                    nc.tensor.matmul(lps[:], h1T[j][:, gb * 128:(gb + 1) * 128],
                                     wpt[j][:], start=(j == 0), stop=(j == HB - 1))
                logits = sm.tile([128, T], F32, tag="logits")
                nc.vector.tensor_tensor(logits[:], lps[:], bpb[:], op=OP.add)
                m = sm.tile([128, 1], F32, tag="m")
                nc.vector.reduce_max(m[:], logits[:], axis=AX.X)
                negm = sm.tile([128, 1], F32, tag="negm")
                nc.scalar.mul(negm[:], m[:], -1.0)
                ex = sm.tile([128, T], F32, tag="ex")
                nc.scalar.activation(ex[:], logits[:], AF.Exp, bias=negm[:])
                ss = sm.tile([128, 1], F32, tag="ss")
                nc.vector.reduce_sum(ss[:], ex[:], axis=AX.X)
                lss = sm.tile([128, 1], F32, tag="lss")
                nc.scalar.activation(lss[:], ss[:], AF.Ln)
                c_t = sm.tile([128, 1], F32, tag="c")
                nc.vector.tensor_add(c_t[:], m[:], lss[:])
                nll = sm.tile([128, T], F32, tag="nll")
                nc.vector.tensor_scalar(nll[:], logits[:], c_t[:], -1.0,
                                        op0=OP.subtract, op1=OP.mult)
                eq = sm.tile([128, T], F32, tag="eq")
                nc.vector.tensor_tensor(eq[:], logits[:],
                                        m[:].broadcast_to([128, T]), op=OP.is_equal)
                sc = sm.tile([128, T], F32, tag="sc")
                nc.vector.tensor_tensor(sc[:], eq[:], rev7[:], op=OP.mult)
                mx = sm.tile([128, 1], F32, tag="mx")
                nc.vector.reduce_max(mx[:], sc[:], axis=AX.X)
                pred = sm.tile([128, 1], F32, tag="pred")
                nc.vector.tensor_scalar(pred[:], mx[:], -1.0, 7.0,
                                        op0=OP.mult, op1=OP.add)
                nc.scalar.copy(table[gb][:, 0:1], pred[:])
                nc.scalar.copy(table[gb][:, 1:1 + T], nll[:])
                nc.scalar.copy(table[gb][:, 1 + T:2 + T], pred[:])

            # ---- phase 7: OT one-hot [g-block partitions, half tokens]
            ot = [wk.tile([128, HALF], F32R, tag=f"ot{gb}", name=f"ot{gb}") for gb in range(2)]
            for gb in range(2):
                nc.vector.tensor_tensor(
                    ot[gb][:], iotag[:, gb:gb + 1].broadcast_to([128, HALF]),
                    indrep[:], op=OP.is_equal)

            # ---- phase 8: token gather + label select
            pred_all = wk.tile([128, HT], F32, tag="pred_all")
            nll_all = wk.tile([128, HT], F32, tag="nll_all")
            for k in range(HT):
                tps = pmm.tile([128, G], F32, tag="mm", name="tps")[:, 0:2 + T]
                ksl = slice(k * 128, (k + 1) * 128)
                nc.tensor.matmul(tps[:], ot[0][:, ksl], table[0][:], start=True, stop=False)
                nc.tensor.matmul(tps[:], ot[1][:, ksl], table[1][:], start=False, stop=True)
                nc.scalar.copy(pred_all[:, k:k + 1], tps[:, 0:1])
                lhot = sm.tile([128, T], F32, tag="lhot")
                nc.vector.tensor_tensor(lhot[:],
                                        labcol[:, k:k + 1].broadcast_to([128, T]),
                                        iota8[:], op=OP.is_equal)
                sel = sm.tile([128, T], F32, tag="sel")
                nc.vector.tensor_tensor(sel[:], tps[:, 1:1 + T], lhot[:], op=OP.mult)
                nc.vector.reduce_sum(nll_all[:, k:k + 1], sel[:], axis=AX.X)
            nc.sync.dma_start(d_pred.ap(), pred_all[:])

            # ---- phase 9: per-partition loss partial sums (host finishes)
            losspair = wk.tile([128, 2], F32, tag="losspair")
            tmp = wk.tile([128, HT], F32, tag="tmpmask")
            nc.vector.tensor_tensor(tmp[:], nll_all[:],
                                    emask[:].broadcast_to([128, HT]), op=OP.mult)
            nc.vector.reduce_sum(losspair[:, 0:1], tmp[:], axis=AX.X)
            tmp2 = wk.tile([128, HT], F32, tag="tmpmask2")
            nc.vector.tensor_tensor(tmp2[:], nll_all[:],
                                    omask[:].broadcast_to([128, HT]), op=OP.mult)
            nc.vector.reduce_sum(losspair[:, 1:2], tmp2[:], axis=AX.X)
            nc.sync.dma_start(d_loss.ap(), losspair[:])

    nc.compile()
    return nc


def _get_nc(nk):
    key = ("nc", nk)
    if key not in _CACHE:
        _CACHE[key] = _build_nc(nk)
    return _CACHE[key]


def kernel(**inputs):
    from concourse.bass_utils import run_bass_kernel_spmd

    E = np.ascontiguousarray(np.asarray(inputs["encoded_states"], dtype=np.float32))
    ind_in = np.asarray(inputs["indicator"])
    lab_in = np.asarray(inputs["ctc_label"])
    W_u = np.asarray(inputs["W_u"], dtype=np.float32)
    b_u = np.asarray(inputs["b_u"], dtype=np.float32)
    W_p = np.asarray(inputs["W_p"], dtype=np.float32)
    b_p = np.asarray(inputs["b_p"], dtype=np.float32)
    ind = ind_in.astype(np.int64)
    lab = lab_in.astype(np.int64)

    nk = S // 128  # 32: full batch per core

    Er = _rnd_fp32r(E.reshape(B * S, H)).reshape(B, S, H)
    wut = _rnd_fp32r(np.ascontiguousarray(W_u.T))
    wpt = _rnd_fp32r(np.ascontiguousarray(W_p.T))

    iotab = np.broadcast_to(np.arange(G, dtype=np.float32)[None, :], (128, G))
    iotag = (np.arange(128, dtype=np.float32)[:, None]
             + np.array([0.0, 128.0], np.float32)[None, :])
    iota8 = np.broadcast_to(np.arange(T, dtype=np.float32)[None, :], (128, T))
    rev7 = np.broadcast_to((7.0 - np.arange(T, dtype=np.float32))[None, :], (128, T))
    bpbc = np.broadcast_to(b_p[None, :], (128, T))
    emask = (np.arange(128) % 2 == 0).astype(np.float32)[:, None]
    omask = (np.arange(128) % 2 == 1).astype(np.float32)[:, None]
    ident = np.eye(128, dtype=np.float32)
    buc = np.broadcast_to(b_u.reshape(HB, 128).T, (128, HB))  # col j = b_u[j*128:(j+1)*128]

    in_maps = []
    for c in range(N_CORES):
        b, h = c // 2, c % 2
        half = slice(h * HALF, (h + 1) * HALF)
        cnt = np.bincount(ind[b], minlength=G)
        invc = (1.0 / np.maximum(cnt, 1)).astype(np.float32)
        indcol = ind[b].astype(np.float32).reshape(nk, 128).T
        labcol = lab[b, half].astype(np.float32).reshape(HT, 128).T
        cpk = np.concatenate([
            indcol, labcol, iotab, iotag, iota8, rev7, bpbc, emask, omask,
            invc.reshape(2, 128).T, buc, ident,
        ], axis=1).astype(np.float32)
        m = {
            "e": Er[b],
            "cpk": np.ascontiguousarray(cpk),
            "indrep": np.broadcast_to(
                ind[b, half].astype(np.float32)[None, :], (128, HALF)).copy(),
            "wut": wut, "wpt": wpt,
        }
        in_maps.append(m)

    nc = _get_nc(nk)
    res = run_bass_kernel_spmd(nc, in_maps, core_ids=list(range(N_CORES)),
                               trace=TRACE)
    _CACHE["last_result"] = res

    pred_f = np.zeros((B, S), np.float32)
    loss = np.zeros(2, np.float64)
    for c in range(N_CORES):
        b, h = c // 2, c % 2
        half = slice(h * HALF, (h + 1) * HALF)
        pred_f[b, half] = res.results[c]["out_pred"].T.reshape(HALF)
        loss += res.results[c]["out_loss"].astype(np.float64).sum(axis=0)

    idt = np.int64 if lab_in.dtype == np.int64 else np.int32
    flatp = pred_f.reshape(-1)
    sep_pred = flatp[0::2].astype(idt)
    tok_pred = flatp[1::2].astype(idt)
    flatl = lab_in.reshape(-1)
    sep_lab = flatl[0::2].copy()
    tok_lab = flatl[1::2].copy()
    sep_loss = np.float32(loss[0] / (B * S / 2))
    tok_loss = np.float32(loss[1] / (B * S / 2))
    return ((sep_loss, sep_pred, sep_lab), (tok_loss, tok_pred, tok_lab))


# revision 8
# speedup vs baseline: 1.0703x; 1.0703x over previous
"""Trainium2 Bass kernel for nn_CtcHead (segment-reduce + 2-layer head + CE).

Sharding: 8 cores, core c -> (batch b = c//2, half h = c%2).
Key algebraic reduction: cell_states[b,s] = group_mean[b, indicator[b,s]]
with only 256 groups per batch, so the tanh-head/log-softmax/argmax run on
256 group rows per batch instead of 16384 tokens.  Per-token outputs are
gathers from a [256, 9] table via a one-hot matmul.

Matmuls run as fp32r (fp32 with 11-bit mantissa, round-to-nearest-even,
exact PSUM f32 accumulate).  Host pre-rounds float inputs to the fp32r
grid so tiles can be DMA'd directly into float32r SBUF tensors.
"""
import sys
import numpy as np

sys.path.insert(0, "/opt/trn_rl_repo")

B, S, H = 4, 4096, 768
G = 256          # indicator groups
T = 8            # NUM_TYPES
HALF = S // 2    # tokens per core
HT = HALF // 128  # token s-blocks per core (16)
HB = H // 128    # h-blocks (6)
N_CORES = 8

_CACHE = {}
TRACE = False


def _rnd_fp32r(x):
    """Round float32 array to the fp32r grid (11-bit mantissa, nearest-even)."""
    u = np.ascontiguousarray(x, np.float32).view(np.uint32).astype(np.uint64)
    lsb = (u >> np.uint64(12)) & np.uint64(1)
    u2 = (u + np.uint64(0x7FF) + lsb) & np.uint64(0xFFFFF000)
    return u2.astype(np.uint32).view(np.float32)


def _build_nc(nk):
    """Build the Bass program.  nk = number of 128-token k-tiles fed to the
    segment-sum (32: full batch per core)."""
    import concourse.bacc as bacc
    import concourse.tile as tile
    import concourse.mybir as mybir

    F32 = mybir.dt.float32
    F32R = mybir.dt.float32r
    AF = mybir.ActivationFunctionType
    OP = mybir.AluOpType
    AX = mybir.AxisListType

    nc = bacc.Bacc("TRN2", target_bir_lowering=False, debug=False,
                   num_devices=N_CORES)

    # packed small constants, one DMA:
    # cols: indcol(nk) labcol(HT) iotab(G) iotag(2) iota8(T) rev7(T) bpb(T)
    #       emask(1) omask(1) invc(2) bu(HB) ident(128)
    CW = nk + HT + G + 2 + T + T + T + 1 + 1 + 2 + HB + 128
    d_e = nc.dram_tensor("e", [nk * 128, H], F32, kind="ExternalInput")
    d_cpk = nc.dram_tensor("cpk", [128, CW], F32, kind="ExternalInput")
    d_indrep = nc.dram_tensor("indrep", [128, HALF], F32, kind="ExternalInput")
    d_wut = nc.dram_tensor("wut", [H, H], F32, kind="ExternalInput")
    d_wpt = nc.dram_tensor("wpt", [H, T], F32, kind="ExternalInput")

    d_pred = nc.dram_tensor("out_pred", [128, HT], F32, kind="ExternalOutput")
    d_loss = nc.dram_tensor("out_loss", [128, 2], F32, kind="ExternalOutput")

    with tile.TileContext(nc) as tc:
        with (
            tc.tile_pool(name="const", bufs=1) as cp,
            tc.tile_pool(name="eb", bufs=1) as eb,
            tc.tile_pool(name="ob", bufs=4) as ob,
            tc.tile_pool(name="work", bufs=1) as wk,
            tc.tile_pool(name="small", bufs=2) as sm,
            tc.tile_pool(name="pseg", bufs=1, space="PSUM") as pseg,
            tc.tile_pool(name="ptr", bufs=2, space="PSUM") as ptr,
            tc.tile_pool(name="pmm", bufs=2, space="PSUM") as pmm,
        ):
            # ---- constants (one packed DMA + 3 weight DMAs)
            cpk = cp.tile([128, CW], F32, tag="cpk")
            nc.sync.dma_start(cpk[:], d_cpk.ap())
            o0 = 0
            indcol = cpk[:, o0:o0 + nk]; o0 += nk
            labcol = cpk[:, o0:o0 + HT]; o0 += HT
            iotab = cpk[:, o0:o0 + G]; o0 += G
            iotag = cpk[:, o0:o0 + 2]; o0 += 2
            iota8 = cpk[:, o0:o0 + T]; o0 += T
            rev7 = cpk[:, o0:o0 + T]; o0 += T
            bpb = cpk[:, o0:o0 + T]; o0 += T
            emask = cpk[:, o0:o0 + 1]; o0 += 1
            omask = cpk[:, o0:o0 + 1]; o0 += 1
            invc = [cpk[:, o0 + gb:o0 + gb + 1] for gb in range(2)]; o0 += 2
            bu = [cpk[:, o0 + j:o0 + j + 1] for j in range(HB)]; o0 += HB
            ident = cpk[:, o0:o0 + 128]; o0 += 128

            indrep = cp.tile([128, HALF], F32, tag="indrep")
            nc.sync.dma_start(indrep[:], d_indrep.ap())
            wut_all = cp.tile([128, HB, H], F32R, tag="wut_all")
            nc.sync.dma_start(
                wut_all[:], d_wut.ap().rearrange("(j p) h -> p j h", p=128).bitcast(F32R))
            wut = [wut_all[:, j, :] for j in range(HB)]
            wpt_all = cp.tile([128, HB, T], F32R, tag="wpt_all")
            nc.sync.dma_start(
                wpt_all[:], d_wpt.ap().rearrange("(j p) t -> p j t", p=128).bitcast(F32R))
            wpt = [wpt_all[:, j, :] for j in range(HB)]

            # ---- phase 1: segment sums  (PSUM [g-block, 768] over nk k-tiles)
            psA = [pseg.tile([128, 512], F32, tag=f"psA{gb}", name=f"psA{gb}") for gb in range(2)]
            psB = [pseg.tile([128, 256], F32, tag=f"psB{gb}", name=f"psB{gb}") for gb in range(2)]
            # e layout: s = p*nk + q  (partition-outer => 1 descriptor per
            # partition row-span; indcol on the host uses the same layout)
            CH = 8  # k-tiles per e-chunk DMA
            e_all = eb.tile([128, nk, H], F32R, tag="e_all")
            e_view = d_e.ap().rearrange("(p q) h -> p q h", p=128)
            for c in range(nk // CH):
                nc.sync.dma_start(
                    e_all[:, c * CH:(c + 1) * CH, :],
                    e_view[:, c * CH:(c + 1) * CH, :].bitcast(F32R))
            for k in range(nk):
                e_t = e_all[:, k, :]
                o_t = ob.tile([128, G], F32R, tag="o")
                nc.vector.tensor_tensor(
                    o_t[:], indcol[:, k:k + 1].broadcast_to([128, G]), iotab[:],
                    op=OP.is_equal)
                st, sp = (k == 0), (k == nk - 1)
                for gb in range(2):
                    osl = o_t[:, gb * 128:(gb + 1) * 128]
                    nc.tensor.matmul(psA[gb][:], osl, e_t[:, 0:512], start=st, stop=sp)
                    nc.tensor.matmul(psB[gb][:], osl, e_t[:, 512:H], start=st, stop=sp)

            # ---- phase 2: means [g, hin] f32  (scale by 1/count)
            means = [wk.tile([128, H], F32, tag=f"means{gb}", name=f"means{gb}") for gb in range(2)]
            for gb in range(2):
                nc.scalar.mul(means[gb][:, 0:512], psA[gb][:], invc[gb][:])
                nc.scalar.mul(means[gb][:, 512:H], psB[gb][:], invc[gb][:])

            # ---- phase 3: transpose -> meansT [hin, g] F32R
            meansT = [wk.tile([128, G], F32R, tag=f"meansT{hb}", name=f"meansT{hb}") for hb in range(HB)]
            for hb in range(HB):
                for gb in range(2):
                    tp = ptr.tile([128, 128], F32, tag="tp")
                    nc.tensor.transpose(tp[:], means[gb][:, hb * 128:(hb + 1) * 128],
                                        ident[:])
                    nc.scalar.copy(meansT[hb][:, gb * 128:(gb + 1) * 128], tp[:])

            # ---- phase 4: L1  h1T[hout-block] = tanh(wut.T @ meansT + bu)
            h1T = [wk.tile([128, G], F32R, tag=f"h1T{j}", name=f"h1T{j}") for j in range(HB)]
            for j in range(HB):
                h1ps = pmm.tile([128, G], F32, tag="mm")
                for hb in range(HB):
                    nc.tensor.matmul(h1ps[:], wut[hb][:, j * 128:(j + 1) * 128],
                                     meansT[hb][:], start=(hb == 0), stop=(hb == HB - 1))
                nc.scalar.activation(h1T[j][:], h1ps[:], AF.Tanh, bias=bu[j][:])

            # ---- phase 5: L2 logits [g-block, 8] + softmax/argmax -> table
            table = [wk.tile([128, 2 + T], F32R, tag=f"table{gb}", name=f"table{gb}") for gb in range(2)]
            for gb in range(2):
                lps = pmm.tile([128, G], F32, tag="mm", name="lps")[:, 0:T]
                for j in range(HB):
                    nc.tensor.matmul(lps[:], h1T[j][:, gb * 128:(gb + 1) * 128],
                                     wpt[j][:], start=(j == 0), stop=(j == HB - 1))
                logits = sm.tile([128, T], F32, tag="logits")
                nc.vector.tensor_tensor(logits[:], lps[:], bpb[:], op=OP.add)
                m = sm.tile([128, 1], F32, tag="m")
                nc.vector.reduce_max(m[:], logits[:], axis=AX.X)
                negm = sm.tile([128, 1], F32, tag="negm")
                nc.scalar.mul(negm[:], m[:], -1.0)
                ex = sm.tile([128, T], F32, tag="ex")
                nc.scalar.activation(ex[:], logits[:], AF.Exp, bias=negm[:])
                ss = sm.tile([128, 1], F32, tag="ss")
                nc.vector.reduce_sum(ss[:], ex[:], axis=AX.X)
                lss = sm.tile([128, 1], F32, tag="lss")
                nc.scalar.activation(lss[:], ss[:], AF.Ln)
                c_t = sm.tile([128, 1], F32, tag="c")
                nc.vector.tensor_add(c_t[:], m[:], lss[:])
                nll = sm.tile([128, T], F32, tag="nll")
                nc.vector.tensor_scalar(nll[:], logits[:], c_t[:], -1.0,
                                        op0=OP.subtract, op1=OP.mult)
                eq = sm.tile([128, T], F32, tag="eq")
                nc.vector.tensor_tensor(eq[:], logits[:],
                                        m[:].broadcast_to([128, T]), op=OP.is_equal)
                sc = sm.tile([128, T], F32, tag="sc")
                nc.vector.tensor_tensor(sc[:], eq[:], rev7[:], op=OP.mult)
                mx = sm.tile([128, 1], F32, tag="mx")
                nc.vector.reduce_max(mx[:], sc[:], axis=AX.X)
                pred = sm.tile([128, 1], F32, tag="pred")
                nc.vector.tensor_scalar(pred[:], mx[:], -1.0, 7.0,
                                        op0=OP.mult, op1=OP.add)
                nc.scalar.copy(table[gb][:, 0:1], pred[:])
                nc.scalar.copy(table[gb][:, 1:1 + T], nll[:])
                nc.scalar.copy(table[gb][:, 1 + T:2 + T], pred[:])

            # ---- phase 7: OT one-hot [g-block partitions, half tokens]
            ot = [wk.tile([128, HALF], F32R, tag=f"ot{gb}", name=f"ot{gb}") for gb in range(2)]
            for gb in range(2):
                nc.vector.tensor_tensor(
                    ot[gb][:], iotag[:, gb:gb + 1].broadcast_to([128, HALF]),
                    indrep[:], op=OP.is_equal)

            # ---- phase 8: token gather + label select
            pred_all = wk.tile([128, HT], F32, tag="pred_all")
            nll_all = wk.tile([128, HT], F32, tag="nll_all")
            for k in range(HT):
                tps = pmm.tile([128, G], F32, tag="mm", name="tps")[:, 0:2 + T]
                ksl = slice(k * 128, (k + 1) * 128)
                nc.tensor.matmul(tps[:], ot[0][:, ksl], table[0][:], start=True, stop=False)
                nc.tensor.matmul(tps[:], ot[1][:, ksl], table[1][:], start=False, stop=True)
                nc.scalar.copy(pred_all[:, k:k + 1], tps[:, 0:1])
                lhot = sm.tile([128, T], F32, tag="lhot")
                nc.vector.tensor_tensor(lhot[:],
                                        labcol[:, k:k + 1].broadcast_to([128, T]),
                                        iota8[:], op=OP.is_equal)
                sel = sm.tile([128, T], F32, tag="sel")
                nc.vector.tensor_tensor(sel[:], tps[:, 1:1 + T], lhot[:], op=OP.mult)
                nc.vector.reduce_sum(nll_all[:, k:k + 1], sel[:], axis=AX.X)
            nc.sync.dma_start(d_pred.ap(), pred_all[:])

            # ---- phase 9: per-partition loss partial sums (host finishes)
            losspair = wk.tile([128, 2], F32, tag="losspair")
            tmp = wk.tile([128, HT], F32, tag="tmpmask")
            nc.vector.tensor_tensor(tmp[:], nll_all[:],
                                    emask[:].broadcast_to([128, HT]), op=OP.mult)
            nc.vector.reduce_sum(losspair[:, 0:1], tmp[:], axis=AX.X)
            tmp2 = wk.tile([128, HT], F32, tag="tmpmask2")
            nc.vector.tensor_tensor(tmp2[:], nll_all[:],
                                    omask[:].broadcast_to([128, HT]), op=OP.mult)
            nc.vector.reduce_sum(losspair[:, 1:2], tmp2[:], axis=AX.X)
            nc.sync.dma_start(d_loss.ap(), losspair[:])

    nc.compile()
    return nc


def _get_nc(nk):
    key = ("nc", nk)
    if key not in _CACHE:
        _CACHE[key] = _build_nc(nk)
    return _CACHE[key]


def kernel(**inputs):
    from concourse.bass_utils import run_bass_kernel_spmd

    E = np.ascontiguousarray(np.asarray(inputs["encoded_states"], dtype=np.float32))
    ind_in = np.asarray(inputs["indicator"])
    lab_in = np.asarray(inputs["ctc_label"])
    W_u = np.asarray(inputs["W_u"], dtype=np.float32)
    b_u = np.asarray(inputs["b_u"], dtype=np.float32)
    W_p = np.asarray(inputs["W_p"], dtype=np.float32)
    b_p = np.asarray(inputs["b_p"], dtype=np.float32)
    ind = ind_in.astype(np.int64)
    lab = lab_in.astype(np.int64)

    nk = S // 128  # 32: full batch per core

    Er = _rnd_fp32r(E.reshape(B * S, H)).reshape(B, S, H)
    wut = _rnd_fp32r(np.ascontiguousarray(W_u.T))
    wpt = _rnd_fp32r(np.ascontiguousarray(W_p.T))

    iotab = np.broadcast_to(np.arange(G, dtype=np.float32)[None, :], (128, G))
    iotag = (np.arange(128, dtype=np.float32)[:, None]
             + np.array([0.0, 128.0], np.float32)[None, :])
    iota8 = np.broadcast_to(np.arange(T, dtype=np.float32)[None, :], (128, T))
    rev7 = np.broadcast_to((7.0 - np.arange(T, dtype=np.float32))[None, :], (128, T))
    bpbc = np.broadcast_to(b_p[None, :], (128, T))
    emask = (np.arange(128) % 2 == 0).astype(np.float32)[:, None]
    omask = (np.arange(128) % 2 == 1).astype(np.float32)[:, None]
    ident = np.eye(128, dtype=np.float32)
    buc = np.broadcast_to(b_u.reshape(HB, 128).T, (128, HB))  # col j = b_u[j*128:(j+1)*128]

    in_maps = []
    for c in range(N_CORES):
        b, h = c // 2, c % 2
        half = slice(h * HALF, (h + 1) * HALF)
        cnt = np.bincount(ind[b], minlength=G)
        invc = (1.0 / np.maximum(cnt, 1)).astype(np.float32)
        indcol = ind[b].astype(np.float32).reshape(128, nk)
        labcol = lab[b, half].astype(np.float32).reshape(HT, 128).T
        cpk = np.concatenate([
            indcol, labcol, iotab, iotag, iota8, rev7, bpbc, emask, omask,
            invc.reshape(2, 128).T, buc, ident,
        ], axis=1).astype(np.float32)
        m = {
            "e": Er[b],
            "cpk": np.ascontiguousarray(cpk),
            "indrep": np.broadcast_to(
                ind[b, half].astype(np.float32)[None, :], (128, HALF)).copy(),
            "wut": wut, "wpt": wpt,
        }
        in_maps.append(m)

    nc = _get_nc(nk)
    res = run_bass_kernel_spmd(nc, in_maps, core_ids=list(range(N_CORES)),
                               trace=TRACE)
    _CACHE["last_result"] = res

    pred_f = np.zeros((B, S), np.float32)
    loss = np.zeros(2, np.float64)
    for c in range(N_CORES):
        b, h = c // 2, c % 2
        half = slice(h * HALF, (h + 1) * HALF)
        pred_f[b, half] = res.results[c]["out_pred"].T.reshape(HALF)
        loss += res.results[c]["out_loss"].astype(np.float64).sum(axis=0)

    idt = np.int64 if lab_in.dtype == np.int64 else np.int32
    flatp = pred_f.reshape(-1)
    sep_pred = flatp[0::2].astype(idt)
    tok_pred = flatp[1::2].astype(idt)
    flatl = lab_in.reshape(-1)
    sep_lab = flatl[0::2].copy()
    tok_lab = flatl[1::2].copy()
    sep_loss = np.float32(loss[0] / (B * S / 2))
    tok_loss = np.float32(loss[1] / (B * S / 2))
    return ((sep_loss, sep_pred, sep_lab), (tok_loss, tok_pred, tok_lab))


# revision 12
# speedup vs baseline: 1.2529x; 1.1706x over previous
"""Trainium2 Bass kernel for nn_CtcHead (segment-reduce + 2-layer head + CE).

Sharding: 8 cores, core c -> (batch b = c//2, half h = c%2).
Key algebraic reduction: cell_states[b,s] = group_mean[b, indicator[b,s]]
with only 256 groups per batch, so the tanh-head/log-softmax/argmax run on
256 group rows per batch instead of 16384 tokens.  Per-token outputs are
gathers from a [256, 9] table via a one-hot matmul.

Matmuls run as fp32r (fp32 with 11-bit mantissa, round-to-nearest-even,
exact PSUM f32 accumulate).  Host pre-rounds float inputs to the fp32r
grid so tiles can be DMA'd directly into float32r SBUF tensors.
"""
import sys
import numpy as np

sys.path.insert(0, "/opt/trn_rl_repo")

B, S, H = 4, 4096, 768
G = 256          # indicator groups
T = 8            # NUM_TYPES
HALF = S // 2    # tokens per core
HT = HALF // 128  # token s-blocks per core (16)
HB = H // 128    # h-blocks (6)
N_CORES = 8

_CACHE = {}
TRACE = False


def _rnd_fp32r(x):
    """Round float32 array to the fp32r grid (11-bit mantissa, nearest-even)."""
    u = np.ascontiguousarray(x, np.float32).view(np.uint32).astype(np.uint64)
    lsb = (u >> np.uint64(12)) & np.uint64(1)
    u2 = (u + np.uint64(0x7FF) + lsb) & np.uint64(0xFFFFF000)
    return u2.astype(np.uint32).view(np.float32)


def _build_nc(nk):
    """Build the Bass program.  nk = number of 128-token k-tiles fed to the
    segment-sum (32: full batch per core)."""
    import concourse.bacc as bacc
    import concourse.tile as tile
    import concourse.mybir as mybir

    F32 = mybir.dt.float32
    F32R = mybir.dt.float32r
    AF = mybir.ActivationFunctionType
    OP = mybir.AluOpType
    AX = mybir.AxisListType

    nc = bacc.Bacc("TRN2", target_bir_lowering=False, debug=False,
                   num_devices=N_CORES)

    # packed small constants, one DMA:
    # cols: indcol(nk) labcol(HT) iotab(G) iotag(2) iota8(T) rev7(T) bpb(T)
    #       emask(1) omask(1) invc(2) bu(HB) ident(128)
    CW = nk + HT + G + 2 + T + T + T + 1 + 1 + 2 + HB + 128
    d_e = nc.dram_tensor("e", [nk * 128, H], F32, kind="ExternalInput")
    d_cpk = nc.dram_tensor("cpk", [128, CW], F32, kind="ExternalInput")
    d_indrep = nc.dram_tensor("indrep", [128, HALF], F32, kind="ExternalInput")
    d_labrep8 = nc.dram_tensor("labrep8", [8, HALF], F32, kind="ExternalInput")
    d_onesr = nc.dram_tensor("onesr", [128, 2], F32, kind="ExternalInput")
    d_wut = nc.dram_tensor("wut", [H, H], F32, kind="ExternalInput")
    d_wpt = nc.dram_tensor("wpt", [H, T], F32, kind="ExternalInput")

    d_pred = nc.dram_tensor("out_pred", [1, HALF], F32, kind="ExternalOutput")
    d_loss = nc.dram_tensor("out_loss", [8, 2 * 4], F32, kind="ExternalOutput")

    with tile.TileContext(nc) as tc:
        with (
            tc.tile_pool(name="const", bufs=1) as cp,
            tc.tile_pool(name="eb", bufs=1) as eb,
            tc.tile_pool(name="ob", bufs=4) as ob,
            tc.tile_pool(name="work", bufs=1) as wk,
            tc.tile_pool(name="small", bufs=2) as sm,
            tc.tile_pool(name="pseg", bufs=1, space="PSUM") as pseg,
            tc.tile_pool(name="ptr", bufs=2, space="PSUM") as ptr,
            tc.tile_pool(name="pmm", bufs=1, space="PSUM") as pmm,
        ):
            # ---- constants (one packed DMA + 3 weight DMAs)
            cpk = cp.tile([128, CW], F32, tag="cpk")
            nc.sync.dma_start(cpk[:], d_cpk.ap())
            o0 = 0
            indcol = cpk[:, o0:o0 + nk]; o0 += nk
            labcol = cpk[:, o0:o0 + HT]; o0 += HT
            iotab = cpk[:, o0:o0 + G]; o0 += G
            iotag = cpk[:, o0:o0 + 2]; o0 += 2
            iota8 = cpk[:, o0:o0 + T]; o0 += T
            rev7 = cpk[:, o0:o0 + T]; o0 += T
            bpb = cpk[:, o0:o0 + T]; o0 += T
            emask = cpk[:, o0:o0 + 1]; o0 += 1
            omask = cpk[:, o0:o0 + 1]; o0 += 1
            invc = [cpk[:, o0 + gb:o0 + gb + 1] for gb in range(2)]; o0 += 2
            bu = [cpk[:, o0 + j:o0 + j + 1] for j in range(HB)]; o0 += HB
            ident = cpk[:, o0:o0 + 128]; o0 += 128


            # ---- phase 1: segment sums  (PSUM [g-block, 768] over nk k-tiles)
            psA = [pseg.tile([128, 512], F32, tag=f"psA{gb}", name=f"psA{gb}") for gb in range(2)]
            psB = [pseg.tile([128, 256], F32, tag=f"psB{gb}", name=f"psB{gb}") for gb in range(2)]
            # e layout: s = p*nk + q  (partition-outer => 1 descriptor per
            # partition row-span; indcol on the host uses the same layout)
            CH = 4  # k-tiles per e-chunk DMA
            e_all = eb.tile([128, nk, H], F32R, tag="e_all")
            e_view = d_e.ap().rearrange("(p q) h -> p q h", p=128)
            for c in range(nk // CH):
                nc.sync.dma_start(
                    e_all[:, c * CH:(c + 1) * CH, :],
                    e_view[:, c * CH:(c + 1) * CH, :].bitcast(F32R))
            # weights and token-stage inputs are needed late; issue after e
            indrep = cp.tile([128, HALF], F32, tag="indrep")
            nc.sync.dma_start(indrep[:], d_indrep.ap())
            labrep8 = cp.tile([8, HALF], F32, tag="labrep8")
            nc.sync.dma_start(labrep8[:], d_labrep8.ap())
            onesr = cp.tile([128, 2], F32R, tag="onesr")
            nc.sync.dma_start(onesr[:], d_onesr.ap().bitcast(F32R))
            wut_all = cp.tile([128, HB, H], F32R, tag="wut_all")
            nc.sync.dma_start(
                wut_all[:], d_wut.ap().rearrange("(j p) h -> p j h", p=128).bitcast(F32R))
            wut = [wut_all[:, j, :] for j in range(HB)]
            wpt_all = cp.tile([128, HB, T], F32R, tag="wpt_all")
            nc.sync.dma_start(
                wpt_all[:], d_wpt.ap().rearrange("(j p) t -> p j t", p=128).bitcast(F32R))
            wpt = [wpt_all[:, j, :] for j in range(HB)]
            for k in range(nk):
                e_t = e_all[:, k, :]
                o_t = ob.tile([128, G], F32R, tag="o")
                nc.vector.tensor_tensor(
                    o_t[:], indcol[:, k:k + 1].broadcast_to([128, G]), iotab[:],
                    op=OP.is_equal)
                st, sp = (k == 0), (k == nk - 1)
                for gb in range(2):
                    osl = o_t[:, gb * 128:(gb + 1) * 128]
                    nc.tensor.matmul(psA[gb][:], osl, e_t[:, 0:512], start=st, stop=sp)
                    nc.tensor.matmul(psB[gb][:], osl, e_t[:, 512:H], start=st, stop=sp)

            # ---- phase 2: means [g, hin] f32  (scale by 1/count)
            means = [wk.tile([128, H], F32, tag=f"means{gb}", name=f"means{gb}") for gb in range(2)]
            for gb in range(2):
                nc.scalar.mul(means[gb][:, 0:512], psA[gb][:], invc[gb][:])
                nc.scalar.mul(means[gb][:, 512:H], psB[gb][:], invc[gb][:])

            # ---- phase 3: transpose -> meansT [hin, g] F32R
            meansT = [wk.tile([128, G], F32R, tag=f"meansT{hb}", name=f"meansT{hb}") for hb in range(HB)]
            for hb in range(HB):
                for gb in range(2):
                    tp = ptr.tile([128, 128], F32, tag="tp")
                    nc.tensor.transpose(tp[:], means[gb][:, hb * 128:(hb + 1) * 128],
                                        ident[:])
                    nc.scalar.copy(meansT[hb][:, gb * 128:(gb + 1) * 128], tp[:])

            # ---- phase 4: L1  h1T[hout-block] = tanh(wut.T @ meansT + bu)
            h1T = [wk.tile([128, G], F32R, tag=f"h1T{j}", name=f"h1T{j}") for j in range(HB)]
            for j in range(HB):
                h1ps = pmm.tile([128, G], F32, tag="mm")
                for hb in range(HB):
                    nc.tensor.matmul(h1ps[:], wut[hb][:, j * 128:(j + 1) * 128],
                                     meansT[hb][:], start=(hb == 0), stop=(hb == HB - 1))
                nc.scalar.activation(h1T[j][:], h1ps[:], AF.Tanh, bias=bu[j][:])

            # ---- phase 5: L2 logits [g-block, 8] + softmax/argmax -> table
            table = [wk.tile([128, 2 + T], F32R, tag=f"table{gb}", name=f"table{gb}") for gb in range(2)]
            for gb in range(2):
                lps = pmm.tile([128, G], F32, tag="mm", name="lps")[:, 0:T]
                for j in range(HB):
                    nc.tensor.matmul(lps[:], h1T[j][:, gb * 128:(gb + 1) * 128],
                                     wpt[j][:], start=(j == 0), stop=(j == HB - 1))
                logits = sm.tile([128, T], F32, tag="logits")
                nc.vector.tensor_tensor(logits[:], lps[:], bpb[:], op=OP.add)
                m = sm.tile([128, 1], F32, tag="m")
                nc.vector.reduce_max(m[:], logits[:], axis=AX.X)
                negm = sm.tile([128, 1], F32, tag="negm")
                nc.scalar.mul(negm[:], m[:], -1.0)
                ex = sm.tile([128, T], F32, tag="ex")
                nc.scalar.activation(ex[:], logits[:], AF.Exp, bias=negm[:])
                ss = sm.tile([128, 1], F32, tag="ss")
                nc.vector.reduce_sum(ss[:], ex[:], axis=AX.X)
                lss = sm.tile([128, 1], F32, tag="lss")
                nc.scalar.activation(lss[:], ss[:], AF.Ln)
                c_t = sm.tile([128, 1], F32, tag="c")
                nc.vector.tensor_add(c_t[:], m[:], lss[:])
                nll = sm.tile([128, T], F32, tag="nll")
                nc.vector.tensor_scalar(nll[:], logits[:], c_t[:], -1.0,
                                        op0=OP.subtract, op1=OP.mult)
                eq = sm.tile([128, T], F32, tag="eq")
                nc.vector.tensor_tensor(eq[:], logits[:],
                                        m[:].broadcast_to([128, T]), op=OP.is_equal)
                sc = sm.tile([128, T], F32, tag="sc")
                nc.vector.tensor_tensor(sc[:], eq[:], rev7[:], op=OP.mult)
                mx = sm.tile([128, 1], F32, tag="mx")
                nc.vector.reduce_max(mx[:], sc[:], axis=AX.X)
                pred = sm.tile([128, 1], F32, tag="pred")
                nc.vector.tensor_scalar(pred[:], mx[:], -1.0, 7.0,
                                        op0=OP.mult, op1=OP.add)
                nc.scalar.copy(table[gb][:, 0:T], nll[:])
                nc.scalar.copy(table[gb][:, T:T + 1], pred[:])
                nc.scalar.copy(table[gb][:, T + 1:T + 2], pred[:])

            # ---- phase 7: OT one-hot [g-block partitions, half tokens]
            ot = [wk.tile([128, HALF], F32R, tag=f"ot{gb}", name=f"ot{gb}") for gb in range(2)]
            for gb in range(2):
                nc.vector.tensor_tensor(
                    ot[gb][:], iotag[:, gb:gb + 1].broadcast_to([128, HALF]),
                    indrep[:], op=OP.is_equal)

            # ---- phase 8: token gather (table stationary, one-hot streams)
            # gps[c] = table.T @ ot[:, chunk]  -> [10, 512]; rows 0..7 = nll
            # per class, row 8 = predicted class, row 9 pad.
            # label select: rows 0..7 * one-hot(labels).T, strided-parity
            # reduction along tokens; partition sum finishes on the host.
            lt8 = wk.tile([8, HALF], F32, tag="lt8")
            nc.vector.tensor_tensor(lt8[:], labrep8[:],
                                    iotag[0:8, 0:1].broadcast_to([8, HALF]),
                                    op=OP.is_equal)
            losspart = wk.tile([8, 2 * 4], F32, tag="losspart")
            predrow = wk.tile([1, HALF], F32, tag="predrow")
            NC4 = HALF // 512
            for c in range(NC4):
                gps = pmm.tile([128, 512], F32, tag="mm", name=f"gps{c}")
                gpp = pmm.tile([128, 512], F32, tag="mmp", name=f"gpp{c}")
                csl = slice(c * 512, (c + 1) * 512)
                nc.tensor.matmul(gps[0:8, :], table[0][:, 0:T], ot[0][:, csl],
                                 start=True, stop=False)
                nc.tensor.matmul(gps[0:8, :], table[1][:, 0:T], ot[1][:, csl],
                                 start=False, stop=True)
                nc.tensor.matmul(gpp[0:2, :], table[0][:, T:T + 2], ot[0][:, csl],
                                 start=True, stop=False)
                nc.tensor.matmul(gpp[0:2, :], table[1][:, T:T + 2], ot[1][:, csl],
                                 start=False, stop=True)
                nc.scalar.copy(predrow[:, csl], gpp[0:1, :])
                prod = sm.tile([8, 512], F32, tag="prod")
                nc.vector.tensor_tensor(prod[:], gps[0:8, :], lt8[:, csl],
                                        op=OP.mult)
                nc.vector.reduce_sum(losspart[:, c:c + 1], prod[:, 0:512:2], axis=AX.X)
                nc.vector.reduce_sum(losspart[:, 4 + c:4 + c + 1], prod[:, 1:512:2],
                                     axis=AX.X)
            nc.sync.dma_start(d_pred.ap(), predrow[:])
            nc.sync.dma_start(d_loss.ap(), losspart[:])

    nc.compile()
    return nc


def _get_nc(nk):
    key = ("nc", nk)
    if key not in _CACHE:
        _CACHE[key] = _build_nc(nk)
    return _CACHE[key]


def kernel(**inputs):
    from concourse.bass_utils import run_bass_kernel_spmd

    E = np.ascontiguousarray(np.asarray(inputs["encoded_states"], dtype=np.float32))
    ind_in = np.asarray(inputs["indicator"])
    lab_in = np.asarray(inputs["ctc_label"])
    W_u = np.asarray(inputs["W_u"], dtype=np.float32)
    b_u = np.asarray(inputs["b_u"], dtype=np.float32)
    W_p = np.asarray(inputs["W_p"], dtype=np.float32)
    b_p = np.asarray(inputs["b_p"], dtype=np.float32)
    ind = ind_in.astype(np.int64)
    lab = lab_in.astype(np.int64)

    nk = S // 128  # 32: full batch per core

    Er = _rnd_fp32r(E.reshape(B * S, H)).reshape(B, S, H)
    wut = _rnd_fp32r(np.ascontiguousarray(W_u.T))
    wpt = _rnd_fp32r(np.ascontiguousarray(W_p.T))

    iotab = np.broadcast_to(np.arange(G, dtype=np.float32)[None, :], (128, G))
    iotag = (np.arange(128, dtype=np.float32)[:, None]
             + np.array([0.0, 128.0], np.float32)[None, :])
    iota8 = np.broadcast_to(np.arange(T, dtype=np.float32)[None, :], (128, T))
    rev7 = np.broadcast_to((7.0 - np.arange(T, dtype=np.float32))[None, :], (128, T))
    bpbc = np.broadcast_to(b_p[None, :], (128, T))
    emask = (np.arange(128) % 2 == 0).astype(np.float32)[:, None]
    omask = (np.arange(128) % 2 == 1).astype(np.float32)[:, None]
    ident = np.eye(128, dtype=np.float32)
    buc = np.broadcast_to(b_u.reshape(HB, 128).T, (128, HB))  # col j = b_u[j*128:(j+1)*128]

    in_maps = []
    for c in range(N_CORES):
        b, h = c // 2, c % 2
        half = slice(h * HALF, (h + 1) * HALF)
        cnt = np.bincount(ind[b], minlength=G)
        invc = (1.0 / np.maximum(cnt, 1)).astype(np.float32)
        indcol = ind[b].astype(np.float32).reshape(128, nk)
        labcol = lab[b, half].astype(np.float32).reshape(HT, 128).T
        cpk = np.concatenate([
            indcol, labcol, iotab, iotag, iota8, rev7, bpbc, emask, omask,
            invc.reshape(2, 128).T, buc, ident,
        ], axis=1).astype(np.float32)
        m = {
            "e": Er[b],
            "cpk": np.ascontiguousarray(cpk),
            "indrep": np.broadcast_to(
                ind[b, half].astype(np.float32)[None, :], (128, HALF)).copy(),
            "wut": wut, "wpt": wpt,
            "labrep8": np.broadcast_to(
                lab[b, half].astype(np.float32)[None, :], (8, HALF)).copy(),
            "onesr": np.ones((128, 2), np.float32),
        }
        in_maps.append(m)

    nc = _get_nc(nk)
    res = run_bass_kernel_spmd(nc, in_maps, core_ids=list(range(N_CORES)),
                               trace=TRACE)
    _CACHE["last_result"] = res

    pred_f = np.zeros((B, S), np.float32)
    loss = np.zeros(2, np.float64)
    for c in range(N_CORES):
        b, h = c // 2, c % 2
        half = slice(h * HALF, (h + 1) * HALF)
        pred_f[b, half] = res.results[c]["out_pred"].reshape(HALF)
        lp = res.results[c]["out_loss"].astype(np.float64)  # [8, 8]: sep c0..3, tok c0..3
        loss[0] += lp[:, 0:4].sum()
        loss[1] += lp[:, 4:8].sum()

    idt = np.int64 if lab_in.dtype == np.int64 else np.int32
    flatp = pred_f.reshape(-1)
    sep_pred = flatp[0::2].astype(idt)
    tok_pred = flatp[1::2].astype(idt)
    flatl = lab_in.reshape(-1)
    sep_lab = flatl[0::2].copy()
    tok_lab = flatl[1::2].copy()
    sep_loss = np.float32(loss[0] / (B * S / 2))
    tok_loss = np.float32(loss[1] / (B * S / 2))
    return ((sep_loss, sep_pred, sep_lab), (tok_loss, tok_pred, tok_lab))
